# revision 14
# baseline (speedup 1.0000x reference)
"""GAT (graph attention) layer on 8 Trainium2 NeuronCores, row-parallel.

out = elu(softmax_row(mask(adj, lrelu(src_i + dst_j))) @ (h @ W))
  with src = (h@W)@a1, dst = (h@W)@a2.

Sharding: each core owns 1024 query rows (rows of the attention matrix);
h/W/a are replicated, adj is row-sharded (host also narrows it to int8 and
pre-transposes h -- pure input marshaling). Inside one core:
  - Wh built from host-pre-transposed hT via PE matmuls (f32r fast path)
  - src for the core's own rows from a host-sliced hTl (full fp32 matmuls)
  - dstb[p, j] = dst_j built during the same hT stream via broadcast-weight
    matmuls (lhsT = w2 replicated along free dim)
  - one fused custom DVE op computes lrelu(dstb + src_i) + (adj*BIG - BIG)
  - ACT exp -> pm (fp16), PE matmul-transposes of pm chunks, aggregation
    matmul pm^T.T @ [Wh | ones] accumulates numerator + row-sums in one
    PSUM tile; normalize by row-sum, elu, DMA out.
"""

import numpy as np

import concourse.bass as bass
import concourse.tile as tile
import concourse.mybir as mybir
from concourse import bacc
from concourse.bass_utils import run_bass_kernel_spmd
from concourse.masks import make_identity

# ---------------- config ----------------
N_NODES, IN_F, OUT_F = 8192, 512, 256
ALPHA = 0.2
BIG = 1.0e30
CORES = 8
R = N_NODES // CORES          # rows per core (1024)
RT = R // 128                 # row-tiles per core (8)
JT = N_NODES // 128           # j-chunks (64)
JS = 2048                     # j-slice for DMA/elementwise
NS = N_NODES // JS            # slices per row-tile (4)
MACRO = 512                   # hT streaming macro tile (nodes)
EM_DT = "f16"                # "f32r" (precise) or "f16" (faster transposes)

f32 = mybir.dt.float32
f32r = mybir.dt.float32r
f16 = mybir.dt.float16
i8 = mybir.dt.int8

# ---------------- custom DVE op ----------------
_REGISTERED = {}


def _get_custom_op():
    if "op" in _REGISTERED:
        return _REGISTERED["op"]
    import concourse.dve_ops as dve_ops
    from concourse.dve_ops import DveOp, _SUB_OPCODE_FOR_NAME
    from concourse.dve_spec import Spec, Src0, Src1, C0, C1, C2, maxx, lower
    from concourse.dve_uop import DveOpSpec

    name = "LRELU_BIAS_MASK_ANT"
    _t = Src0 + C0
    spec = Spec(
        body=maxx(_t, _t * C2) + (Src1 * C1 - C1),
        reference=lambda in0, in1, s0, s1, imm2: (
            np.maximum(in0 + s0, (in0 + s0) * imm2)
            + (in1.astype(np.float32) * s1 - s1)
        ).astype(np.float32),
    )
    if name not in _SUB_OPCODE_FOR_NAME:
        row = max(_SUB_OPCODE_FOR_NAME.values()) + 1
        _SUB_OPCODE_FOR_NAME[name] = row
        tmp = DveOpSpec(name=name, opcode=row, uops=lower(spec, ver="v3"),
                        rd1_en=True)
        op = DveOp(name, spec, subdim=False, uops_sha={"v3": tmp.sha("v3")})
        dve_ops.OPS.append(op)
        dve_ops.CUSTOM_DVE_SPECS[name] = spec
    else:
        op = next(o for o in dve_ops.OPS if o.name == name)
    _REGISTERED["op"] = op
    return op


# ---------------- kernel builder ----------------
_BUILD_CACHE = {}


def _build_nc(debug=False):
    key = "nc_dbg" if debug else "nc"
    if key in _BUILD_CACHE:
        return _BUILD_CACHE[key]
    OP = _get_custom_op()
    AT = mybir.AluOpType
    AF = mybir.ActivationFunctionType

    nc = bacc.Bacc("TRN2", target_bir_lowering=False, debug=False,
                   num_devices=CORES)

    # hT is declared f32r: host sends raw fp32 bytes; the PE rounds when
    # streaming (measured ~1.5e-4 relative on matmul results).
    hT_ext = nc.dram_tensor("hT", [IN_F, N_NODES], f32r, kind="ExternalInput").ap()
    hTl_ext = nc.dram_tensor("hTl", [IN_F, R], f32, kind="ExternalInput").ap()
    adj_ext = nc.dram_tensor("adj", [R, N_NODES], i8, kind="ExternalInput").ap()
    W_ext = nc.dram_tensor("W", [IN_F, OUT_F], f32, kind="ExternalInput").ap()
    Wt_ext = nc.dram_tensor("Wt", [OUT_F, IN_F], f32, kind="ExternalInput").ap()
    a12_ext = nc.dram_tensor("a12", [OUT_F, 2], f32, kind="ExternalInput").ap()
    out_ext = nc.dram_tensor("out", [R, OUT_F], f32, kind="ExternalOutput").ap()
    if debug:
        dbg_dstb = nc.dram_tensor("dbg_dstb", [128, 512], f32, kind="ExternalOutput").ap()
        dbg_srcl = nc.dram_tensor("dbg_srcl", [128, 8], f32, kind="ExternalOutput").ap()
        dbg_whaug = nc.dram_tensor("dbg_whaug", [128, 4 * (OUT_F + 1)], f32, kind="ExternalOutput").ap()
        dbg_em = nc.dram_tensor("dbg_em", [128, 512], f32, kind="ExternalOutput").ap()
        dbg_agg = nc.dram_tensor("dbg_agg", [128, OUT_F + 1], f32, kind="ExternalOutput").ap()

    KT = IN_F // 128  # 4 contraction tiles

    with tile.TileContext(nc) as tc:
        with tc.tile_pool(name="const", bufs=1) as cpool, \
             tc.tile_pool(name="hT", bufs=2 * KT) as hpool, \
             tc.tile_pool(name="whaug", bufs=1) as wapool, \
             tc.tile_pool(name="small", bufs=1) as spool, \
             tc.tile_pool(name="dstb", bufs=1) as dpool, \
             tc.tile_pool(name="adj", bufs=(4 if debug else 6)) as apool, \
             tc.tile_pool(name="em", bufs=3) as empool, \
             tc.tile_pool(name="pmT", bufs=4) as ptpool, \
             tc.tile_pool(name="outp", bufs=2) as opool, \
             tc.tile_pool(name="dbgp", bufs=1) as dbgpool, \
             tc.tile_pool(name="mm_ps", bufs=2, space="PSUM") as mmps, \
             tc.tile_pool(name="tp_ps", bufs=4, space="PSUM") as tpps, \
             tc.tile_pool(name="mi_ps", bufs=2, space="PSUM") as mips:

            # ---- constants (Wt/a12 first: they gate the dstb chain) ----
            Wtk = []
            for k in range(2):
                t = cpool.tile([128, IN_F], f32, tag=f"Wt{k}")
                nc.sync.dma_start(out=t[:], in_=Wt_ext[k * 128:(k + 1) * 128, :])
                Wtk.append(t)
            a12k = []
            for k in range(2):
                t = cpool.tile([128, 2], f32, tag=f"a12_{k}")
                nc.sync.dma_start(out=t[:], in_=a12_ext[k * 128:(k + 1) * 128, :])
                a12k.append(t)
            Wk, Wkr = [], []
            for k in range(KT):
                t = cpool.tile([128, OUT_F], f32, tag=f"W{k}")
                nc.scalar.dma_start(out=t[:], in_=W_ext[k * 128:(k + 1) * 128, :])
                Wk.append(t)
                tr = cpool.tile([128, OUT_F], f32r, tag=f"Wr{k}")
                nc.vector.tensor_copy(tr[:], t[:])
                Wkr.append(tr)
            id16 = cpool.tile([128, 128], f16, tag="id16")
            make_identity(nc, id16[:])
            em_dt = f16 if EM_DT == "f16" else f32r
            if EM_DT == "f16":
                id_em = id16
            else:
                id_em = cpool.tile([128, 128], f32r, tag="id_em")
                nc.vector.tensor_copy(id_em[:], id16[:])

            # hTl tiles (core's own rows, fp32 exact)
            hTl = []
            for k in range(KT):
                t = cpool.tile([128, R], f32, tag=f"hTl{k}")
                nc.scalar.dma_start(out=t[:], in_=hTl_ext[k * 128:(k + 1) * 128, :])
                hTl.append(t)

            # ---- w1w2[feat, 2] = [W@a1 | W@a2] ----
            w12 = []
            for ftile in range(KT):
                ps = mips.tile([128, 512], f32, tag="mi")
                for k in range(2):
                    nc.tensor.matmul(ps[:, 0:2],
                                     Wtk[k][:, ftile * 128:(ftile + 1) * 128],
                                     a12k[k][:], start=(k == 0), stop=(k == 1))
                t = cpool.tile([128, 2], f32, tag=f"w12_{ftile}")
                nc.vector.tensor_copy(t[:], ps[:, 0:2])
                w12.append(t)
            # w2 replicated along free dim (f32r) for dstb broadcast matmuls
            w2bc = []
            for k in range(KT):
                t = cpool.tile([128, 128], f32r, tag=f"w2bc{k}")
                nc.vector.tensor_copy(t[:], w12[k][:, 1:2].broadcast_to([128, 128]))
                w2bc.append(t)

            # ---- src_local[p, t] = src for the core's own row-tiles (fp32) ----
            src_local = spool.tile([128, 8], f32, tag="src_local")
            slps = mips.tile([128, 512], f32, tag="mi")
            for t in range(RT):
                for k in range(KT):
                    nc.tensor.matmul(slps[:, t:t + 1],
                                     hTl[k][:, t * 128:(t + 1) * 128],
                                     w12[k][:, 0:1],
                                     start=(k == 0), stop=(k == KT - 1))
            nc.vector.tensor_copy(src_local[:], slps[:, 0:8])

            # ---- stream hT: Wh(+fp16 cast) and dstb blocks ----
            whaug = wapool.tile([128, JT * (OUT_F + 1)], f16, tag="whaug")
            wh3 = whaug[:].rearrange("p (c w) -> p c w", w=OUT_F + 1)
            nc.vector.memset(wh3[:, :, OUT_F:OUT_F + 1], 1.0)
            dstb = dpool.tile([128, N_NODES], f32, tag="dstb")

            for im in range(N_NODES // MACRO):
                hkr = []
                for k in range(KT):
                    t = hpool.tile([128, MACRO], f32r, tag="hT")
                    nc.sync.dma_start(
                        out=t[:],
                        in_=hT_ext[k * 128:(k + 1) * 128,
                                   im * MACRO:(im + 1) * MACRO])
                    hkr.append(t)
                # dstb block for this macro
                dps = mips.tile([128, 512], f32, tag="mi")
                for k in range(KT):
                    nc.tensor.matmul(dps[:], w2bc[k][:], hkr[k][:],
                                     start=(k == 0), stop=(k == KT - 1))
                nc.vector.tensor_copy(
                    dstb[:, im * MACRO:(im + 1) * MACRO], dps[:])
                # Wh for the macro's i-tiles
                for it in range(MACRO // 128):
                    g = im * (MACRO // 128) + it
                    sl = slice(it * 128, (it + 1) * 128)
                    wps = mmps.tile([128, OUT_F + 1], f32, tag="mm")
                    for k in range(KT):
                        nc.tensor.matmul(wps[:, 0:OUT_F], hkr[k][:, sl],
                                         Wkr[k][:],
                                         start=(k == 0), stop=(k == KT - 1))
                    if g % 2 == 0:
                        nc.vector.tensor_copy(wh3[:, g, 0:OUT_F], wps[:, 0:OUT_F])
                    else:
                        nc.scalar.copy(wh3[:, g, 0:OUT_F], wps[:, 0:OUT_F])

            if debug:
                nc.sync.dma_start(out=dbg_srcl[:], in_=src_local[:])
                wtmp = dbgpool.tile([128, 4 * (OUT_F + 1)], f32, tag="wtmp")
                nc.vector.tensor_copy(wtmp[:], whaug[:, 0:4 * (OUT_F + 1)])
                nc.sync.dma_start(out=dbg_whaug[:], in_=wtmp[:])
                nc.sync.dma_start(out=dbg_dstb[:], in_=dstb[:, 0:512])

            # ---- attention row-tiles ----
            for t in range(RT):
                aps = mmps.tile([128, OUT_F + 1], f32, tag="mm")
                for s in range(NS):
                    adj_t = apool.tile([128, JS], i8, tag="adj")
                    nc.scalar.dma_start(
                        out=adj_t[:],
                        in_=adj_ext[t * 128:(t + 1) * 128, s * JS:(s + 1) * JS])
                    em_t = empool.tile([128, JS], em_dt, tag="em")
                    nc.vector._custom_dve(OP, out=em_t[:],
                                          in0=dstb[:, s * JS:(s + 1) * JS],
                                          in1=adj_t[:],
                                          s0=src_local[:, t:t + 1],
                                          s1=BIG, imm2=ALPHA)
                    if debug and t == 0 and s == 0:
                        nc.sync.dma_start(out=dbg_em[:],
                                          in_=em_t[:, 0:512].bitcast(f32) if EM_DT == "f32r" else em_t[:, 0:512])
                    for q in range(JS // 512):
                        tp = tpps.tile([128, 512], em_dt, tag="tp")
                        for u in range(4):
                            nc.tensor.matmul(
                                tp[:, u * 128:(u + 1) * 128],
                                em_t[:, (q * 4 + u) * 128:(q * 4 + u + 1) * 128],
                                id_em[:], is_transpose=True,
                                start=(u == 0), stop=(u == 3))
                        c4 = ptpool.tile([128, 512], f16, tag="pmT")
                        nc.scalar.activation(c4[:], tp[:], AF.Exp)
                        for u in range(4):
                            c = s * (JS // 128) + q * 4 + u
                            nc.tensor.matmul(
                                aps[:],
                                c4[:, u * 128:(u + 1) * 128],
                                wh3[:, c, :],
                                start=(c == 0), stop=(c == JT - 1))
                if debug and t == 0:
                    atmp = dbgpool.tile([128, OUT_F + 1], f32, tag="atmp")
                    nc.vector.tensor_copy(atmp[:], aps[:])
                    nc.sync.dma_start(out=dbg_agg[:], in_=atmp[:])
                # normalize + elu: out = relu(x) - 1 + exp(min(x, 0)), x = num/den
                sumc = opool.tile([128, 1], f32, tag="sumc")
                nc.vector.tensor_copy(sumc[:], aps[:, OUT_F:OUT_F + 1])
                rec = opool.tile([128, 1], f32, tag="rec")
                nc.vector.reciprocal(rec[:], sumc[:])
                r1 = opool.tile([128, OUT_F], f32, tag="r1")
                nc.vector.tensor_scalar(r1[:], aps[:, 0:OUT_F], rec[:], 0.0,
                                        AT.mult, AT.max)
                xm = opool.tile([128, OUT_F], f32, tag="xm")
                nc.vector.tensor_scalar(xm[:], aps[:, 0:OUT_F], rec[:], 0.0,
                                        AT.mult, AT.min)
                qe = opool.tile([128, OUT_F], f32, tag="qe")
                nc.scalar.activation(qe[:], xm[:], AF.Exp)
                elu = opool.tile([128, OUT_F], f32, tag="elu")
                nc.vector.scalar_tensor_tensor(elu[:], r1[:], -1.0, qe[:],
                                               AT.add, AT.add)
                nc.sync.dma_start(out=out_ext[t * 128:(t + 1) * 128, :],
                                  in_=elu[:])

    nc.finalize()
    _BUILD_CACHE[key] = nc
    return nc


def kernel(h, adj, W, a1, a2):
    h = np.asarray(h, dtype=np.float32)
    W = np.asarray(W, dtype=np.float32)
    a1 = np.asarray(a1, dtype=np.float32)
    a2 = np.asarray(a2, dtype=np.float32)

    nc = _build_nc()

    hT = np.ascontiguousarray(h.T)
    adj8 = np.asarray(adj, dtype=np.int8)
    Wt = np.ascontiguousarray(W.T)
    a12 = np.ascontiguousarray(np.stack([a1, a2], axis=1))

    in_maps = []
    for c in range(CORES):
        in_maps.append({
            "hT": hT,
            "hTl": np.ascontiguousarray(hT[:, c * R:(c + 1) * R]),
            "adj": adj8[c * R:(c + 1) * R, :],
            "W": W,
            "Wt": Wt,
            "a12": a12,
        })
    res = run_bass_kernel_spmd(nc, in_maps, list(range(CORES)))
    out = np.concatenate([res.results[c]["out"] for c in range(CORES)], axis=0)
    return out


# revision 15
# speedup vs baseline: 1.0538x; 1.0538x over previous
"""GAT (graph attention) layer on 8 Trainium2 NeuronCores, row-parallel.

out = elu(softmax_row(mask(adj, lrelu(src_i + dst_j))) @ (h @ W))
  with src = (h@W)@a1, dst = (h@W)@a2.

Sharding: each core owns 1024 query rows (rows of the attention matrix);
h/W/a are replicated, adj is row-sharded (host also narrows it to int8 and
pre-transposes h -- pure input marshaling). Inside one core:
  - Wh built from host-pre-transposed hT via PE matmuls (f32r fast path)
  - src for the core's own rows from a host-sliced hTl (full fp32 matmuls)
  - dstb[p, j] = dst_j built during the same hT stream via broadcast-weight
    matmuls (lhsT = w2 replicated along free dim)
  - one fused custom DVE op computes lrelu(dstb + src_i) + (adj*BIG - BIG)
  - ACT exp -> pm (fp16), PE matmul-transposes of pm chunks, aggregation
    matmul pm^T.T @ [Wh | ones] accumulates numerator + row-sums in one
    PSUM tile; normalize by row-sum, elu, DMA out.
"""

import numpy as np

import concourse.bass as bass
import concourse.tile as tile
import concourse.mybir as mybir
from concourse import bacc
from concourse.bass_utils import run_bass_kernel_spmd
from concourse.masks import make_identity

# ---------------- config ----------------
N_NODES, IN_F, OUT_F = 8192, 512, 256
ALPHA = 0.2
BIG = 1.0e30
CORES = 8
R = N_NODES // CORES          # rows per core (1024)
RT = R // 128                 # row-tiles per core (8)
JT = N_NODES // 128           # j-chunks (64)
JS = 2048                     # j-slice for DMA/elementwise
NS = N_NODES // JS            # slices per row-tile (4)
MACRO = 512                   # hT streaming macro tile (nodes)
EM_DT = "f32r"                # "f32r" (precise) or "f16" (faster transposes)

f32 = mybir.dt.float32
f32r = mybir.dt.float32r
f16 = mybir.dt.float16
i8 = mybir.dt.int8

# ---------------- custom DVE op ----------------
_REGISTERED = {}


def _get_custom_op():
    if "op" in _REGISTERED:
        return _REGISTERED["op"]
    import concourse.dve_ops as dve_ops
    from concourse.dve_ops import DveOp, _SUB_OPCODE_FOR_NAME
    from concourse.dve_spec import Spec, Src0, Src1, C0, C1, C2, maxx, lower
    from concourse.dve_uop import DveOpSpec

    name = "LRELU_BIAS_MASK_ANT"
    _t = Src0 + C0
    spec = Spec(
        body=maxx(_t, _t * C2) + (Src1 * C1 - C1),
        reference=lambda in0, in1, s0, s1, imm2: (
            np.maximum(in0 + s0, (in0 + s0) * imm2)
            + (in1.astype(np.float32) * s1 - s1)
        ).astype(np.float32),
    )
    if name not in _SUB_OPCODE_FOR_NAME:
        row = max(_SUB_OPCODE_FOR_NAME.values()) + 1
        _SUB_OPCODE_FOR_NAME[name] = row
        tmp = DveOpSpec(name=name, opcode=row, uops=lower(spec, ver="v3"),
                        rd1_en=True)
        op = DveOp(name, spec, subdim=False, uops_sha={"v3": tmp.sha("v3")})
        dve_ops.OPS.append(op)
        dve_ops.CUSTOM_DVE_SPECS[name] = spec
    else:
        op = next(o for o in dve_ops.OPS if o.name == name)
    _REGISTERED["op"] = op
    return op


# ---------------- kernel builder ----------------
_BUILD_CACHE = {}


def _build_nc(debug=False):
    key = "nc_dbg" if debug else "nc"
    if key in _BUILD_CACHE:
        return _BUILD_CACHE[key]
    OP = _get_custom_op()
    AT = mybir.AluOpType
    AF = mybir.ActivationFunctionType

    nc = bacc.Bacc("TRN2", target_bir_lowering=False, debug=False,
                   num_devices=CORES)

    # hT is declared f32r: host sends raw fp32 bytes; the PE rounds when
    # streaming (measured ~1.5e-4 relative on matmul results).
    hT_ext = nc.dram_tensor("hT", [IN_F, N_NODES], f32r, kind="ExternalInput").ap()
    hTl_ext = nc.dram_tensor("hTl", [IN_F, R], f32, kind="ExternalInput").ap()
    adj_ext = nc.dram_tensor("adj", [R, N_NODES], i8, kind="ExternalInput").ap()
    W_ext = nc.dram_tensor("W", [IN_F, OUT_F], f32, kind="ExternalInput").ap()
    Wt_ext = nc.dram_tensor("Wt", [OUT_F, IN_F], f32, kind="ExternalInput").ap()
    a12_ext = nc.dram_tensor("a12", [OUT_F, 2], f32, kind="ExternalInput").ap()
    out_ext = nc.dram_tensor("out", [R, OUT_F], f32, kind="ExternalOutput").ap()
    if debug:
        dbg_dstb = nc.dram_tensor("dbg_dstb", [128, 512], f32, kind="ExternalOutput").ap()
        dbg_srcl = nc.dram_tensor("dbg_srcl", [128, 8], f32, kind="ExternalOutput").ap()
        dbg_whaug = nc.dram_tensor("dbg_whaug", [128, 4 * (OUT_F + 1)], f32, kind="ExternalOutput").ap()
        dbg_em = nc.dram_tensor("dbg_em", [128, 512], f32, kind="ExternalOutput").ap()
        dbg_agg = nc.dram_tensor("dbg_agg", [128, OUT_F + 1], f32, kind="ExternalOutput").ap()

    KT = IN_F // 128  # 4 contraction tiles

    with tile.TileContext(nc) as tc:
        with tc.tile_pool(name="const", bufs=1) as cpool, \
             tc.tile_pool(name="hT", bufs=2 * KT) as hpool, \
             tc.tile_pool(name="whaug", bufs=1) as wapool, \
             tc.tile_pool(name="small", bufs=1) as spool, \
             tc.tile_pool(name="dstb", bufs=1) as dpool, \
             tc.tile_pool(name="adj", bufs=(4 if debug else 6)) as apool, \
             tc.tile_pool(name="em", bufs=4) as empool, \
             tc.tile_pool(name="pmT", bufs=8) as ptpool, \
             tc.tile_pool(name="outp", bufs=2) as opool, \
             tc.tile_pool(name="dbgp", bufs=1) as dbgpool, \
             tc.tile_pool(name="mm_ps", bufs=2, space="PSUM") as mmps, \
             tc.tile_pool(name="tp_ps", bufs=5, space="PSUM") as tpps, \
             tc.tile_pool(name="mi_ps", bufs=1, space="PSUM") as mips:

            # ---- constants (Wt/a12 first: they gate the dstb chain) ----
            Wtk = []
            for k in range(2):
                t = cpool.tile([128, IN_F], f32, tag=f"Wt{k}")
                nc.sync.dma_start(out=t[:], in_=Wt_ext[k * 128:(k + 1) * 128, :])
                Wtk.append(t)
            a12k = []
            for k in range(2):
                t = cpool.tile([128, 2], f32, tag=f"a12_{k}")
                nc.sync.dma_start(out=t[:], in_=a12_ext[k * 128:(k + 1) * 128, :])
                a12k.append(t)
            Wk, Wkr = [], []
            for k in range(KT):
                t = cpool.tile([128, OUT_F], f32, tag=f"W{k}")
                nc.scalar.dma_start(out=t[:], in_=W_ext[k * 128:(k + 1) * 128, :])
                Wk.append(t)
                tr = cpool.tile([128, OUT_F], f32r, tag=f"Wr{k}")
                nc.vector.tensor_copy(tr[:], t[:])
                Wkr.append(tr)
            id16 = cpool.tile([128, 128], f16, tag="id16")
            make_identity(nc, id16[:])
            em_dt = f16 if EM_DT == "f16" else f32r
            if EM_DT == "f16":
                id_em = id16
            else:
                id_em = cpool.tile([128, 128], f32r, tag="id_em")
                nc.vector.tensor_copy(id_em[:], id16[:])

            # hTl tiles (core's own rows, fp32 exact)
            hTl = []
            for k in range(KT):
                t = cpool.tile([128, R], f32, tag=f"hTl{k}")
                nc.scalar.dma_start(out=t[:], in_=hTl_ext[k * 128:(k + 1) * 128, :])
                hTl.append(t)

            # ---- w1w2[feat, 2] = [W@a1 | W@a2] ----
            w12 = []
            for ftile in range(KT):
                ps = mips.tile([128, 512], f32, tag="mi")
                for k in range(2):
                    nc.tensor.matmul(ps[:, 0:2],
                                     Wtk[k][:, ftile * 128:(ftile + 1) * 128],
                                     a12k[k][:], start=(k == 0), stop=(k == 1))
                t = cpool.tile([128, 2], f32, tag=f"w12_{ftile}")
                nc.vector.tensor_copy(t[:], ps[:, 0:2])
                w12.append(t)
            # w2 replicated along free dim (f32r) for dstb broadcast matmuls
            w2bc = []
            for k in range(KT):
                t = cpool.tile([128, 128], f32r, tag=f"w2bc{k}")
                nc.vector.tensor_copy(t[:], w12[k][:, 1:2].broadcast_to([128, 128]))
                w2bc.append(t)

            # ---- src_local[p, t] = src for the core's own row-tiles (fp32) ----
            src_local = spool.tile([128, 8], f32, tag="src_local")
            slps = mips.tile([128, 512], f32, tag="mi")
            for t in range(RT):
                for k in range(KT):
                    nc.tensor.matmul(slps[:, t:t + 1],
                                     hTl[k][:, t * 128:(t + 1) * 128],
                                     w12[k][:, 0:1],
                                     start=(k == 0), stop=(k == KT - 1))
            nc.vector.tensor_copy(src_local[:], slps[:, 0:8])

            # ---- stream hT: Wh(+fp16 cast) and dstb blocks ----
            whaug = wapool.tile([128, JT * (OUT_F + 1)], f16, tag="whaug")
            wh3 = whaug[:].rearrange("p (c w) -> p c w", w=OUT_F + 1)
            nc.vector.memset(wh3[:, :, OUT_F:OUT_F + 1], 1.0)
            dstb = dpool.tile([128, N_NODES], f32, tag="dstb")

            for im in range(N_NODES // MACRO):
                hkr = []
                for k in range(KT):
                    t = hpool.tile([128, MACRO], f32r, tag="hT")
                    nc.sync.dma_start(
                        out=t[:],
                        in_=hT_ext[k * 128:(k + 1) * 128,
                                   im * MACRO:(im + 1) * MACRO])
                    hkr.append(t)
                # dstb block for this macro
                dps = mips.tile([128, 512], f32, tag="mi")
                for k in range(KT):
                    nc.tensor.matmul(dps[:], w2bc[k][:], hkr[k][:],
                                     start=(k == 0), stop=(k == KT - 1))
                nc.vector.tensor_copy(
                    dstb[:, im * MACRO:(im + 1) * MACRO], dps[:])
                # Wh for the macro's i-tiles
                for it in range(MACRO // 128):
                    g = im * (MACRO // 128) + it
                    sl = slice(it * 128, (it + 1) * 128)
                    wps = mmps.tile([128, OUT_F + 1], f32, tag="mm")
                    for k in range(KT):
                        nc.tensor.matmul(wps[:, 0:OUT_F], hkr[k][:, sl],
                                         Wkr[k][:],
                                         start=(k == 0), stop=(k == KT - 1))
                    if g % 2 == 0:
                        nc.vector.tensor_copy(wh3[:, g, 0:OUT_F], wps[:, 0:OUT_F])
                    else:
                        nc.scalar.copy(wh3[:, g, 0:OUT_F], wps[:, 0:OUT_F])

            if debug:
                nc.sync.dma_start(out=dbg_srcl[:], in_=src_local[:])
                wtmp = dbgpool.tile([128, 4 * (OUT_F + 1)], f32, tag="wtmp")
                nc.vector.tensor_copy(wtmp[:], whaug[:, 0:4 * (OUT_F + 1)])
                nc.sync.dma_start(out=dbg_whaug[:], in_=wtmp[:])
                nc.sync.dma_start(out=dbg_dstb[:], in_=dstb[:, 0:512])

            # ---- attention row-tiles ----
            for t in range(RT):
                aps = mmps.tile([128, OUT_F + 1], f32, tag="mm")
                for s in range(NS):
                    adj_t = apool.tile([128, JS], i8, tag="adj")
                    nc.scalar.dma_start(
                        out=adj_t[:],
                        in_=adj_ext[t * 128:(t + 1) * 128, s * JS:(s + 1) * JS])
                    em_t = empool.tile([128, JS], em_dt, tag="em")
                    for he in range(2):
                        hs = slice(he * (JS // 2), (he + 1) * (JS // 2))
                        nc.vector._custom_dve(OP, out=em_t[:, hs],
                                              in0=dstb[:, s * JS + he * (JS // 2):
                                                       s * JS + (he + 1) * (JS // 2)],
                                              in1=adj_t[:, hs],
                                              s0=src_local[:, t:t + 1],
                                              s1=BIG, imm2=ALPHA)
                    if debug and t == 0 and s == 0:
                        nc.sync.dma_start(out=dbg_em[:],
                                          in_=em_t[:, 0:512].bitcast(f32) if EM_DT == "f32r" else em_t[:, 0:512])
                    for q in range(JS // 512):
                        tp = tpps.tile([128, 512], em_dt, tag="tp")
                        for u in range(4):
                            nc.tensor.matmul(
                                tp[:, u * 128:(u + 1) * 128],
                                em_t[:, (q * 4 + u) * 128:(q * 4 + u + 1) * 128],
                                id_em[:], is_transpose=True,
                                start=(u == 0), stop=(u == 3))
                        c4 = ptpool.tile([128, 512], f16, tag="pmT")
                        nc.scalar.activation(c4[:], tp[:], AF.Exp)
                        for u in range(4):
                            c = s * (JS // 128) + q * 4 + u
                            nc.tensor.matmul(
                                aps[:],
                                c4[:, u * 128:(u + 1) * 128],
                                wh3[:, c, :],
                                start=(c == 0), stop=(c == JT - 1))
                if debug and t == 0:
                    atmp = dbgpool.tile([128, OUT_F + 1], f32, tag="atmp")
                    nc.vector.tensor_copy(atmp[:], aps[:])
                    nc.sync.dma_start(out=dbg_agg[:], in_=atmp[:])
                # normalize + elu: out = relu(x) - 1 + exp(min(x, 0)), x = num/den
                sumc = opool.tile([128, 1], f32, tag="sumc")
                nc.vector.tensor_copy(sumc[:], aps[:, OUT_F:OUT_F + 1])
                rec = opool.tile([128, 1], f32, tag="rec")
                nc.vector.reciprocal(rec[:], sumc[:])
                r1 = opool.tile([128, OUT_F], f32, tag="r1")
                nc.vector.tensor_scalar(r1[:], aps[:, 0:OUT_F], rec[:], 0.0,
                                        AT.mult, AT.max)
                xm = opool.tile([128, OUT_F], f32, tag="xm")
                nc.vector.tensor_scalar(xm[:], aps[:, 0:OUT_F], rec[:], 0.0,
                                        AT.mult, AT.min)
                qe = opool.tile([128, OUT_F], f32, tag="qe")
                nc.scalar.activation(qe[:], xm[:], AF.Exp)
                elu = opool.tile([128, OUT_F], f32, tag="elu")
                nc.vector.scalar_tensor_tensor(elu[:], r1[:], -1.0, qe[:],
                                               AT.add, AT.add)
                nc.sync.dma_start(out=out_ext[t * 128:(t + 1) * 128, :],
                                  in_=elu[:])

    nc.finalize()
    _BUILD_CACHE[key] = nc
    return nc


def kernel(h, adj, W, a1, a2):
    h = np.asarray(h, dtype=np.float32)
    W = np.asarray(W, dtype=np.float32)
    a1 = np.asarray(a1, dtype=np.float32)
    a2 = np.asarray(a2, dtype=np.float32)

    nc = _build_nc()

    hT = np.ascontiguousarray(h.T)
    adj8 = np.asarray(adj, dtype=np.int8)
    Wt = np.ascontiguousarray(W.T)
    a12 = np.ascontiguousarray(np.stack([a1, a2], axis=1))

    in_maps = []
    for c in range(CORES):
        in_maps.append({
            "hT": hT,
            "hTl": np.ascontiguousarray(hT[:, c * R:(c + 1) * R]),
            "adj": adj8[c * R:(c + 1) * R, :],
            "W": W,
            "Wt": Wt,
            "a12": a12,
        })
    res = run_bass_kernel_spmd(nc, in_maps, list(range(CORES)))
    out = np.concatenate([res.results[c]["out"] for c in range(CORES)], axis=0)
    return out


# revision 16
# speedup vs baseline: 1.0549x; 1.0011x over previous
"""GAT (graph attention) layer on 8 Trainium2 NeuronCores, row-parallel.

out = elu(softmax_row(mask(adj, lrelu(src_i + dst_j))) @ (h @ W))
  with src = (h@W)@a1, dst = (h@W)@a2.

Sharding: each core owns 1024 query rows (rows of the attention matrix);
h/W/a are replicated, adj is row-sharded (host also narrows it to int8 and
pre-transposes h -- pure input marshaling). Inside one core:
  - Wh built from host-pre-transposed hT via PE matmuls (f32r fast path)
  - src for the core's own rows from a host-sliced hTl (full fp32 matmuls)
  - dstb[p, j] = dst_j built during the same hT stream via broadcast-weight
    matmuls (lhsT = w2 replicated along free dim)
  - one fused custom DVE op computes lrelu(dstb + src_i) + (adj*BIG - BIG)
  - ACT exp -> pm (fp16), PE matmul-transposes of pm chunks, aggregation
    matmul pm^T.T @ [Wh | ones] accumulates numerator + row-sums in one
    PSUM tile; normalize by row-sum, elu, DMA out.
"""

import numpy as np

import concourse.bass as bass
import concourse.tile as tile
import concourse.mybir as mybir
from concourse import bacc
from concourse.bass_utils import run_bass_kernel_spmd
from concourse.masks import make_identity

# ---------------- config ----------------
N_NODES, IN_F, OUT_F = 8192, 512, 256
ALPHA = 0.2
BIG = 1.0e30
CORES = 8
R = N_NODES // CORES          # rows per core (1024)
RT = R // 128                 # row-tiles per core (8)
JT = N_NODES // 128           # j-chunks (64)
JS = 2048                     # j-slice for DMA/elementwise
NS = N_NODES // JS            # slices per row-tile (4)
MACRO = 512                   # hT streaming macro tile (nodes)
EM_DT = "f32r"                # "f32r" (precise) or "f16" (faster transposes)

f32 = mybir.dt.float32
f32r = mybir.dt.float32r
f16 = mybir.dt.float16
i8 = mybir.dt.int8

# ---------------- custom DVE op ----------------
_REGISTERED = {}


def _get_custom_op():
    if "op" in _REGISTERED:
        return _REGISTERED["op"]
    import concourse.dve_ops as dve_ops
    from concourse.dve_ops import DveOp, _SUB_OPCODE_FOR_NAME
    from concourse.dve_spec import Spec, Src0, Src1, C0, C1, C2, maxx, lower
    from concourse.dve_uop import DveOpSpec

    name = "LRELU_BIAS_MASK_ANT"
    _t = Src0 + C0
    spec = Spec(
        body=maxx(_t, _t * C2) + (Src1 * C1 - C1),
        reference=lambda in0, in1, s0, s1, imm2: (
            np.maximum(in0 + s0, (in0 + s0) * imm2)
            + (in1.astype(np.float32) * s1 - s1)
        ).astype(np.float32),
    )
    if name not in _SUB_OPCODE_FOR_NAME:
        row = max(_SUB_OPCODE_FOR_NAME.values()) + 1
        _SUB_OPCODE_FOR_NAME[name] = row
        tmp = DveOpSpec(name=name, opcode=row, uops=lower(spec, ver="v3"),
                        rd1_en=True)
        op = DveOp(name, spec, subdim=False, uops_sha={"v3": tmp.sha("v3")})
        dve_ops.OPS.append(op)
        dve_ops.CUSTOM_DVE_SPECS[name] = spec
    else:
        op = next(o for o in dve_ops.OPS if o.name == name)
    _REGISTERED["op"] = op
    return op


# ---------------- kernel builder ----------------
_BUILD_CACHE = {}


def _build_nc(debug=False):
    key = "nc_dbg" if debug else "nc"
    if key in _BUILD_CACHE:
        return _BUILD_CACHE[key]
    OP = _get_custom_op()
    AT = mybir.AluOpType
    AF = mybir.ActivationFunctionType

    nc = bacc.Bacc("TRN2", target_bir_lowering=False, debug=False,
                   num_devices=CORES)

    # hT is declared f32r: host sends raw fp32 bytes; the PE rounds when
    # streaming (measured ~1.5e-4 relative on matmul results).
    hT_ext = nc.dram_tensor("hT", [IN_F, N_NODES], f32r, kind="ExternalInput").ap()
    hTl_ext = nc.dram_tensor("hTl", [IN_F, R], f32, kind="ExternalInput").ap()
    adj_ext = nc.dram_tensor("adj", [R, N_NODES], i8, kind="ExternalInput").ap()
    W_ext = nc.dram_tensor("W", [IN_F, OUT_F], f32, kind="ExternalInput").ap()
    Wt_ext = nc.dram_tensor("Wt", [OUT_F, IN_F], f32, kind="ExternalInput").ap()
    a12_ext = nc.dram_tensor("a12", [OUT_F, 2], f32, kind="ExternalInput").ap()
    out_ext = nc.dram_tensor("out", [R, OUT_F], f32, kind="ExternalOutput").ap()
    if debug:
        dbg_dstb = nc.dram_tensor("dbg_dstb", [128, 512], f32, kind="ExternalOutput").ap()
        dbg_srcl = nc.dram_tensor("dbg_srcl", [128, 8], f32, kind="ExternalOutput").ap()
        dbg_whaug = nc.dram_tensor("dbg_whaug", [128, 4 * (OUT_F + 1)], f32, kind="ExternalOutput").ap()
        dbg_em = nc.dram_tensor("dbg_em", [128, 512], f32, kind="ExternalOutput").ap()
        dbg_agg = nc.dram_tensor("dbg_agg", [128, OUT_F + 1], f32, kind="ExternalOutput").ap()

    KT = IN_F // 128  # 4 contraction tiles

    with tile.TileContext(nc) as tc:
        with tc.tile_pool(name="const", bufs=1) as cpool, \
             tc.tile_pool(name="hT", bufs=2 * KT) as hpool, \
             tc.tile_pool(name="whaug", bufs=1) as wapool, \
             tc.tile_pool(name="small", bufs=1) as spool, \
             tc.tile_pool(name="dstb", bufs=1) as dpool, \
             tc.tile_pool(name="adj", bufs=(4 if debug else 6)) as apool, \
             tc.tile_pool(name="em", bufs=4) as empool, \
             tc.tile_pool(name="pmT", bufs=8) as ptpool, \
             tc.tile_pool(name="outp", bufs=2) as opool, \
             tc.tile_pool(name="dbgp", bufs=1) as dbgpool, \
             tc.tile_pool(name="mm_ps", bufs=2, space="PSUM") as mmps, \
             tc.tile_pool(name="tp_ps", bufs=5, space="PSUM") as tpps, \
             tc.tile_pool(name="mi_ps", bufs=1, space="PSUM") as mips:

            # ---- constants (Wt/a12 first: they gate the dstb chain) ----
            Wtk = []
            for k in range(2):
                t = cpool.tile([128, IN_F], f32, tag=f"Wt{k}")
                nc.sync.dma_start(out=t[:], in_=Wt_ext[k * 128:(k + 1) * 128, :])
                Wtk.append(t)
            a12k = []
            for k in range(2):
                t = cpool.tile([128, 2], f32, tag=f"a12_{k}")
                nc.sync.dma_start(out=t[:], in_=a12_ext[k * 128:(k + 1) * 128, :])
                a12k.append(t)
            Wk, Wkr = [], []
            for k in range(KT):
                t = cpool.tile([128, OUT_F], f32, tag=f"W{k}")
                nc.scalar.dma_start(out=t[:], in_=W_ext[k * 128:(k + 1) * 128, :])
                Wk.append(t)
                tr = cpool.tile([128, OUT_F], f32r, tag=f"Wr{k}")
                nc.vector.tensor_copy(tr[:], t[:])
                Wkr.append(tr)
            id16 = cpool.tile([128, 128], f16, tag="id16")
            make_identity(nc, id16[:])
            em_dt = f16 if EM_DT == "f16" else f32r
            if EM_DT == "f16":
                id_em = id16
            else:
                id_em = cpool.tile([128, 128], f32r, tag="id_em")
                nc.vector.tensor_copy(id_em[:], id16[:])

            # hTl tiles (core's own rows, fp32 exact)
            hTl = []
            for k in range(KT):
                t = cpool.tile([128, R], f32, tag=f"hTl{k}")
                nc.scalar.dma_start(out=t[:], in_=hTl_ext[k * 128:(k + 1) * 128, :])
                hTl.append(t)

            # ---- w1w2[feat, 2] = [W@a1 | W@a2] ----
            w12 = []
            for ftile in range(KT):
                ps = mips.tile([128, 512], f32, tag="mi")
                for k in range(2):
                    nc.tensor.matmul(ps[:, 0:2],
                                     Wtk[k][:, ftile * 128:(ftile + 1) * 128],
                                     a12k[k][:], start=(k == 0), stop=(k == 1))
                t = cpool.tile([128, 2], f32, tag=f"w12_{ftile}")
                nc.vector.tensor_copy(t[:], ps[:, 0:2])
                w12.append(t)
            # w2 replicated along free dim (f32r) for dstb broadcast matmuls
            w2bc = []
            for k in range(KT):
                t = cpool.tile([128, 128], f32r, tag=f"w2bc{k}")
                nc.vector.tensor_copy(t[:], w12[k][:, 1:2].broadcast_to([128, 128]))
                w2bc.append(t)

            # ---- src_local[p, t] = src for the core's own row-tiles (fp32) ----
            src_local = spool.tile([128, 8], f32, tag="src_local")
            slps = mips.tile([128, 512], f32, tag="mi")
            for t in range(RT):
                for k in range(KT):
                    nc.tensor.matmul(slps[:, t:t + 1],
                                     hTl[k][:, t * 128:(t + 1) * 128],
                                     w12[k][:, 0:1],
                                     start=(k == 0), stop=(k == KT - 1))
            nc.vector.tensor_copy(src_local[:], slps[:, 0:8])

            # ---- stream hT: Wh(+fp16 cast) and dstb blocks ----
            whaug = wapool.tile([128, JT * (OUT_F + 1)], f16, tag="whaug")
            wh3 = whaug[:].rearrange("p (c w) -> p c w", w=OUT_F + 1)
            nc.vector.memset(wh3[:, :, OUT_F:OUT_F + 1], 1.0)
            dstb = dpool.tile([128, N_NODES], f32, tag="dstb")

            for im in range(N_NODES // MACRO):
                hkr = []
                for k in range(KT):
                    t = hpool.tile([128, MACRO], f32r, tag="hT")
                    nc.sync.dma_start(
                        out=t[:],
                        in_=hT_ext[k * 128:(k + 1) * 128,
                                   im * MACRO:(im + 1) * MACRO])
                    hkr.append(t)
                # dstb block for this macro
                dps = mips.tile([128, 512], f32, tag="mi")
                for k in range(KT):
                    nc.tensor.matmul(dps[:], w2bc[k][:], hkr[k][:],
                                     start=(k == 0), stop=(k == KT - 1))
                nc.vector.tensor_copy(
                    dstb[:, im * MACRO:(im + 1) * MACRO], dps[:])
                # Wh for the macro's i-tiles
                for it in range(MACRO // 128):
                    g = im * (MACRO // 128) + it
                    sl = slice(it * 128, (it + 1) * 128)
                    wps = mmps.tile([128, OUT_F + 1], f32, tag="mm")
                    for k in range(KT):
                        nc.tensor.matmul(wps[:, 0:OUT_F], hkr[k][:, sl],
                                         Wkr[k][:],
                                         start=(k == 0), stop=(k == KT - 1))
                    if g % 2 == 0:
                        nc.vector.tensor_copy(wh3[:, g, 0:OUT_F], wps[:, 0:OUT_F])
                    else:
                        nc.scalar.copy(wh3[:, g, 0:OUT_F], wps[:, 0:OUT_F])

            if debug:
                nc.sync.dma_start(out=dbg_srcl[:], in_=src_local[:])
                wtmp = dbgpool.tile([128, 4 * (OUT_F + 1)], f32, tag="wtmp")
                nc.vector.tensor_copy(wtmp[:], whaug[:, 0:4 * (OUT_F + 1)])
                nc.sync.dma_start(out=dbg_whaug[:], in_=wtmp[:])
                nc.sync.dma_start(out=dbg_dstb[:], in_=dstb[:, 0:512])

            # ---- attention row-tiles ----
            for t in range(RT):
                aps = mmps.tile([128, OUT_F + 1], f32, tag="mm")
                for s in range(NS):
                    adj_t = apool.tile([128, JS], i8, tag="adj")
                    nc.scalar.dma_start(
                        out=adj_t[:],
                        in_=adj_ext[t * 128:(t + 1) * 128, s * JS:(s + 1) * JS])
                    em_t = empool.tile([128, JS], em_dt, tag="em")
                    for he in range(2):
                        hs = slice(he * (JS // 2), (he + 1) * (JS // 2))
                        nc.vector._custom_dve(OP, out=em_t[:, hs],
                                              in0=dstb[:, s * JS + he * (JS // 2):
                                                       s * JS + (he + 1) * (JS // 2)],
                                              in1=adj_t[:, hs],
                                              s0=src_local[:, t:t + 1],
                                              s1=BIG, imm2=ALPHA)
                    if debug and t == 0 and s == 0:
                        nc.sync.dma_start(out=dbg_em[:],
                                          in_=em_t[:, 0:512].bitcast(f32) if EM_DT == "f32r" else em_t[:, 0:512])
                    c4s = []
                    for q in range(JS // 512):
                        tp = tpps.tile([128, 512], em_dt, tag="tp")
                        for u in range(4):
                            nc.tensor.matmul(
                                tp[:, u * 128:(u + 1) * 128],
                                em_t[:, (q * 4 + u) * 128:(q * 4 + u + 1) * 128],
                                id_em[:], is_transpose=True,
                                start=(u == 0), stop=(u == 3))
                        c4 = ptpool.tile([128, 512], f16, tag="pmT")
                        nc.scalar.activation(c4[:], tp[:], AF.Exp)
                        c4s.append(c4)
                    for q in range(JS // 512):
                        for u in range(4):
                            c = s * (JS // 128) + q * 4 + u
                            nc.tensor.matmul(
                                aps[:],
                                c4s[q][:, u * 128:(u + 1) * 128],
                                wh3[:, c, :],
                                start=(c == 0), stop=(c == JT - 1))
                if debug and t == 0:
                    atmp = dbgpool.tile([128, OUT_F + 1], f32, tag="atmp")
                    nc.vector.tensor_copy(atmp[:], aps[:])
                    nc.sync.dma_start(out=dbg_agg[:], in_=atmp[:])
                # normalize + elu: out = relu(x) - 1 + exp(min(x, 0)), x = num/den
                sumc = opool.tile([128, 1], f32, tag="sumc")
                nc.vector.tensor_copy(sumc[:], aps[:, OUT_F:OUT_F + 1])
                rec = opool.tile([128, 1], f32, tag="rec")
                nc.vector.reciprocal(rec[:], sumc[:])
                r1 = opool.tile([128, OUT_F], f32, tag="r1")
                nc.vector.tensor_scalar(r1[:], aps[:, 0:OUT_F], rec[:], 0.0,
                                        AT.mult, AT.max)
                xm = opool.tile([128, OUT_F], f32, tag="xm")
                nc.vector.tensor_scalar(xm[:], aps[:, 0:OUT_F], rec[:], 0.0,
                                        AT.mult, AT.min)
                qe = opool.tile([128, OUT_F], f32, tag="qe")
                nc.scalar.activation(qe[:], xm[:], AF.Exp)
                elu = opool.tile([128, OUT_F], f32, tag="elu")
                nc.vector.scalar_tensor_tensor(elu[:], r1[:], -1.0, qe[:],
                                               AT.add, AT.add)
                nc.sync.dma_start(out=out_ext[t * 128:(t + 1) * 128, :],
                                  in_=elu[:])

    nc.finalize()
    _BUILD_CACHE[key] = nc
    return nc


def kernel(h, adj, W, a1, a2):
    h = np.asarray(h, dtype=np.float32)
    W = np.asarray(W, dtype=np.float32)
    a1 = np.asarray(a1, dtype=np.float32)
    a2 = np.asarray(a2, dtype=np.float32)

    nc = _build_nc()

    hT = np.ascontiguousarray(h.T)
    adj8 = np.asarray(adj, dtype=np.int8)
    Wt = np.ascontiguousarray(W.T)
    a12 = np.ascontiguousarray(np.stack([a1, a2], axis=1))

    in_maps = []
    for c in range(CORES):
        in_maps.append({
            "hT": hT,
            "hTl": np.ascontiguousarray(hT[:, c * R:(c + 1) * R]),
            "adj": adj8[c * R:(c + 1) * R, :],
            "W": W,
            "Wt": Wt,
            "a12": a12,
        })
    res = run_bass_kernel_spmd(nc, in_maps, list(range(CORES)))
    out = np.concatenate([res.results[c]["out"] for c in range(CORES)], axis=0)
    return out


# revision 17
# speedup vs baseline: 1.0746x; 1.0187x over previous
"""GAT (graph attention) layer on 8 Trainium2 NeuronCores, row-parallel.

out = elu(softmax_row(mask(adj, lrelu(src_i + dst_j))) @ (h @ W))
  with src = (h@W)@a1, dst = (h@W)@a2.

Sharding: each core owns 1024 query rows (rows of the attention matrix);
h/W/a are replicated, adj is row-sharded (host also narrows it to int8 and
pre-transposes h -- pure input marshaling). Inside one core:
  - Wh built from host-pre-transposed hT via PE matmuls (f32r fast path)
  - src for the core's own rows from a host-sliced hTl (full fp32 matmuls)
  - dstb[p, j] = dst_j built during the same hT stream via broadcast-weight
    matmuls (lhsT = w2 replicated along free dim)
  - one fused custom DVE op computes lrelu(dstb + src_i) + (adj*BIG - BIG)
  - ACT exp -> pm (fp16), PE matmul-transposes of pm chunks, aggregation
    matmul pm^T.T @ [Wh | ones] accumulates numerator + row-sums in one
    PSUM tile; normalize by row-sum, elu, DMA out.
"""

import numpy as np

import concourse.bass as bass
import concourse.tile as tile
import concourse.mybir as mybir
from concourse import bacc
from concourse.bass_utils import run_bass_kernel_spmd
from concourse.masks import make_identity

# ---------------- config ----------------
N_NODES, IN_F, OUT_F = 8192, 512, 256
ALPHA = 0.2
BIG = 1.0e30
CORES = 8
R = N_NODES // CORES          # rows per core (1024)
RT = R // 128                 # row-tiles per core (8)
JT = N_NODES // 128           # j-chunks (64)
JS = 2048                     # j-slice for DMA/elementwise
NS = N_NODES // JS            # slices per row-tile (4)
MACRO = 512                   # hT streaming macro tile (nodes)
EM_DT = "f16"                # "f32r" (precise) or "f16" (faster transposes)

f32 = mybir.dt.float32
f32r = mybir.dt.float32r
f16 = mybir.dt.float16
i8 = mybir.dt.int8

# ---------------- custom DVE op ----------------
_REGISTERED = {}


def _get_custom_op():
    if "op" in _REGISTERED:
        return _REGISTERED["op"]
    import concourse.dve_ops as dve_ops
    from concourse.dve_ops import DveOp, _SUB_OPCODE_FOR_NAME
    from concourse.dve_spec import Spec, Src0, Src1, C0, C1, C2, maxx, lower
    from concourse.dve_uop import DveOpSpec

    name = "LRELU_BIAS_MASK_ANT"
    _t = Src0 + C0
    spec = Spec(
        body=maxx(_t, _t * C2) + (Src1 * C1 - C1),
        reference=lambda in0, in1, s0, s1, imm2: (
            np.maximum(in0 + s0, (in0 + s0) * imm2)
            + (in1.astype(np.float32) * s1 - s1)
        ).astype(np.float32),
    )
    if name not in _SUB_OPCODE_FOR_NAME:
        row = max(_SUB_OPCODE_FOR_NAME.values()) + 1
        _SUB_OPCODE_FOR_NAME[name] = row
        tmp = DveOpSpec(name=name, opcode=row, uops=lower(spec, ver="v3"),
                        rd1_en=True)
        op = DveOp(name, spec, subdim=False, uops_sha={"v3": tmp.sha("v3")})
        dve_ops.OPS.append(op)
        dve_ops.CUSTOM_DVE_SPECS[name] = spec
    else:
        op = next(o for o in dve_ops.OPS if o.name == name)
    _REGISTERED["op"] = op
    return op


# ---------------- kernel builder ----------------
_BUILD_CACHE = {}


def _build_nc(debug=False):
    key = "nc_dbg" if debug else "nc"
    if key in _BUILD_CACHE:
        return _BUILD_CACHE[key]
    OP = _get_custom_op()
    AT = mybir.AluOpType
    AF = mybir.ActivationFunctionType

    nc = bacc.Bacc("TRN2", target_bir_lowering=False, debug=False,
                   num_devices=CORES)

    # hT is declared f32r: host sends raw fp32 bytes; the PE rounds when
    # streaming (measured ~1.5e-4 relative on matmul results).
    hT_ext = nc.dram_tensor("hT", [IN_F, N_NODES], f32r, kind="ExternalInput").ap()
    hTl_ext = nc.dram_tensor("hTl", [IN_F, R], f32, kind="ExternalInput").ap()
    adj_ext = nc.dram_tensor("adj", [R, N_NODES], i8, kind="ExternalInput").ap()
    W_ext = nc.dram_tensor("W", [IN_F, OUT_F], f32, kind="ExternalInput").ap()
    Wt_ext = nc.dram_tensor("Wt", [OUT_F, IN_F], f32, kind="ExternalInput").ap()
    a12_ext = nc.dram_tensor("a12", [OUT_F, 2], f32, kind="ExternalInput").ap()
    out_ext = nc.dram_tensor("out", [R, OUT_F], f32, kind="ExternalOutput").ap()
    if debug:
        dbg_dstb = nc.dram_tensor("dbg_dstb", [128, 512], f32, kind="ExternalOutput").ap()
        dbg_srcl = nc.dram_tensor("dbg_srcl", [128, 8], f32, kind="ExternalOutput").ap()
        dbg_whaug = nc.dram_tensor("dbg_whaug", [128, 4 * (OUT_F + 1)], f32, kind="ExternalOutput").ap()
        dbg_em = nc.dram_tensor("dbg_em", [128, 512], f32, kind="ExternalOutput").ap()
        dbg_agg = nc.dram_tensor("dbg_agg", [128, OUT_F + 1], f32, kind="ExternalOutput").ap()

    KT = IN_F // 128  # 4 contraction tiles

    with tile.TileContext(nc) as tc:
        with tc.tile_pool(name="const", bufs=1) as cpool, \
             tc.tile_pool(name="hT", bufs=2 * KT) as hpool, \
             tc.tile_pool(name="whaug", bufs=1) as wapool, \
             tc.tile_pool(name="small", bufs=1) as spool, \
             tc.tile_pool(name="dstb", bufs=1) as dpool, \
             tc.tile_pool(name="adj", bufs=(4 if debug else 6)) as apool, \
             tc.tile_pool(name="em", bufs=4) as empool, \
             tc.tile_pool(name="pmT", bufs=8) as ptpool, \
             tc.tile_pool(name="outp", bufs=2) as opool, \
             tc.tile_pool(name="dbgp", bufs=1) as dbgpool, \
             tc.tile_pool(name="mm_ps", bufs=2, space="PSUM") as mmps, \
             tc.tile_pool(name="tp_ps", bufs=5, space="PSUM") as tpps, \
             tc.tile_pool(name="mi_ps", bufs=1, space="PSUM") as mips:

            # ---- constants (Wt/a12 first: they gate the dstb chain) ----
            Wtk = []
            for k in range(2):
                t = cpool.tile([128, IN_F], f32, tag=f"Wt{k}")
                nc.sync.dma_start(out=t[:], in_=Wt_ext[k * 128:(k + 1) * 128, :])
                Wtk.append(t)
            a12k = []
            for k in range(2):
                t = cpool.tile([128, 2], f32, tag=f"a12_{k}")
                nc.sync.dma_start(out=t[:], in_=a12_ext[k * 128:(k + 1) * 128, :])
                a12k.append(t)
            Wk, Wkr = [], []
            for k in range(KT):
                t = cpool.tile([128, OUT_F], f32, tag=f"W{k}")
                nc.scalar.dma_start(out=t[:], in_=W_ext[k * 128:(k + 1) * 128, :])
                Wk.append(t)
                tr = cpool.tile([128, OUT_F], f32r, tag=f"Wr{k}")
                nc.vector.tensor_copy(tr[:], t[:])
                Wkr.append(tr)
            id16 = cpool.tile([128, 128], f16, tag="id16")
            make_identity(nc, id16[:])
            em_dt = f16 if EM_DT == "f16" else f32r
            if EM_DT == "f16":
                id_em = id16
            else:
                id_em = cpool.tile([128, 128], f32r, tag="id_em")
                nc.vector.tensor_copy(id_em[:], id16[:])

            # hTl tiles (core's own rows, fp32 exact)
            hTl = []
            for k in range(KT):
                t = cpool.tile([128, R], f32, tag=f"hTl{k}")
                nc.scalar.dma_start(out=t[:], in_=hTl_ext[k * 128:(k + 1) * 128, :])
                hTl.append(t)

            # ---- w1w2[feat, 2] = [W@a1 | W@a2] ----
            w12 = []
            for ftile in range(KT):
                ps = mips.tile([128, 512], f32, tag="mi")
                for k in range(2):
                    nc.tensor.matmul(ps[:, 0:2],
                                     Wtk[k][:, ftile * 128:(ftile + 1) * 128],
                                     a12k[k][:], start=(k == 0), stop=(k == 1))
                t = cpool.tile([128, 2], f32, tag=f"w12_{ftile}")
                nc.vector.tensor_copy(t[:], ps[:, 0:2])
                w12.append(t)
            # w2 replicated along free dim (f32r) for dstb broadcast matmuls
            w2bc = []
            for k in range(KT):
                t = cpool.tile([128, 128], f32r, tag=f"w2bc{k}")
                nc.vector.tensor_copy(t[:], w12[k][:, 1:2].broadcast_to([128, 128]))
                w2bc.append(t)

            # ---- src_local[p, t] = src for the core's own row-tiles (fp32) ----
            src_local = spool.tile([128, 8], f32, tag="src_local")
            slps = mips.tile([128, 512], f32, tag="mi")
            for t in range(RT):
                for k in range(KT):
                    nc.tensor.matmul(slps[:, t:t + 1],
                                     hTl[k][:, t * 128:(t + 1) * 128],
                                     w12[k][:, 0:1],
                                     start=(k == 0), stop=(k == KT - 1))
            nc.vector.tensor_copy(src_local[:], slps[:, 0:8])

            # ---- stream hT: Wh(+fp16 cast) and dstb blocks ----
            whaug = wapool.tile([128, JT * (OUT_F + 1)], f16, tag="whaug")
            wh3 = whaug[:].rearrange("p (c w) -> p c w", w=OUT_F + 1)
            nc.vector.memset(wh3[:, :, OUT_F:OUT_F + 1], 1.0)
            dstb = dpool.tile([128, N_NODES], f32, tag="dstb")

            for im in range(N_NODES // MACRO):
                hkr = []
                for k in range(KT):
                    t = hpool.tile([128, MACRO], f32r, tag="hT")
                    nc.sync.dma_start(
                        out=t[:],
                        in_=hT_ext[k * 128:(k + 1) * 128,
                                   im * MACRO:(im + 1) * MACRO])
                    hkr.append(t)
                # dstb block for this macro
                dps = mips.tile([128, 512], f32, tag="mi")
                for k in range(KT):
                    nc.tensor.matmul(dps[:], w2bc[k][:], hkr[k][:],
                                     start=(k == 0), stop=(k == KT - 1))
                nc.vector.tensor_copy(
                    dstb[:, im * MACRO:(im + 1) * MACRO], dps[:])
                # Wh for the macro's i-tiles
                for it in range(MACRO // 128):
                    g = im * (MACRO // 128) + it
                    sl = slice(it * 128, (it + 1) * 128)
                    wps = mmps.tile([128, OUT_F + 1], f32, tag="mm")
                    for k in range(KT):
                        nc.tensor.matmul(wps[:, 0:OUT_F], hkr[k][:, sl],
                                         Wkr[k][:],
                                         start=(k == 0), stop=(k == KT - 1))
                    if g % 2 == 0:
                        nc.vector.tensor_copy(wh3[:, g, 0:OUT_F], wps[:, 0:OUT_F])
                    else:
                        nc.scalar.copy(wh3[:, g, 0:OUT_F], wps[:, 0:OUT_F])

            if debug:
                nc.sync.dma_start(out=dbg_srcl[:], in_=src_local[:])
                wtmp = dbgpool.tile([128, 4 * (OUT_F + 1)], f32, tag="wtmp")
                nc.vector.tensor_copy(wtmp[:], whaug[:, 0:4 * (OUT_F + 1)])
                nc.sync.dma_start(out=dbg_whaug[:], in_=wtmp[:])
                nc.sync.dma_start(out=dbg_dstb[:], in_=dstb[:, 0:512])

            # ---- attention row-tiles ----
            for t in range(RT):
                aps = mmps.tile([128, OUT_F + 1], f32, tag="mm")
                for s in range(NS):
                    adj_t = apool.tile([128, JS], i8, tag="adj")
                    nc.scalar.dma_start(
                        out=adj_t[:],
                        in_=adj_ext[t * 128:(t + 1) * 128, s * JS:(s + 1) * JS])
                    em_t = empool.tile([128, JS], em_dt, tag="em")
                    for he in range(2):
                        hs = slice(he * (JS // 2), (he + 1) * (JS // 2))
                        nc.vector._custom_dve(OP, out=em_t[:, hs],
                                              in0=dstb[:, s * JS + he * (JS // 2):
                                                       s * JS + (he + 1) * (JS // 2)],
                                              in1=adj_t[:, hs],
                                              s0=src_local[:, t:t + 1],
                                              s1=BIG, imm2=ALPHA)
                    if debug and t == 0 and s == 0:
                        nc.sync.dma_start(out=dbg_em[:],
                                          in_=em_t[:, 0:512].bitcast(f32) if EM_DT == "f32r" else em_t[:, 0:512])
                    c4s = []
                    for q in range(JS // 512):
                        tp = tpps.tile([128, 512], em_dt, tag="tp")
                        for u in range(4):
                            nc.tensor.matmul(
                                tp[:, u * 128:(u + 1) * 128],
                                em_t[:, (q * 4 + u) * 128:(q * 4 + u + 1) * 128],
                                id_em[:], is_transpose=True,
                                start=(u == 0), stop=(u == 3))
                        c4 = ptpool.tile([128, 512], f16, tag="pmT")
                        nc.scalar.activation(c4[:], tp[:], AF.Exp)
                        c4s.append(c4)
                    for q in range(JS // 512):
                        for u in range(4):
                            c = s * (JS // 128) + q * 4 + u
                            nc.tensor.matmul(
                                aps[:],
                                c4s[q][:, u * 128:(u + 1) * 128],
                                wh3[:, c, :],
                                start=(c == 0), stop=(c == JT - 1))
                if debug and t == 0:
                    atmp = dbgpool.tile([128, OUT_F + 1], f32, tag="atmp")
                    nc.vector.tensor_copy(atmp[:], aps[:])
                    nc.sync.dma_start(out=dbg_agg[:], in_=atmp[:])
                # normalize + elu: out = relu(x) - 1 + exp(min(x, 0)), x = num/den
                sumc = opool.tile([128, 1], f32, tag="sumc")
                nc.vector.tensor_copy(sumc[:], aps[:, OUT_F:OUT_F + 1])
                rec = opool.tile([128, 1], f32, tag="rec")
                nc.vector.reciprocal(rec[:], sumc[:])
                r1 = opool.tile([128, OUT_F], f32, tag="r1")
                nc.vector.tensor_scalar(r1[:], aps[:, 0:OUT_F], rec[:], 0.0,
                                        AT.mult, AT.max)
                xm = opool.tile([128, OUT_F], f32, tag="xm")
                nc.vector.tensor_scalar(xm[:], aps[:, 0:OUT_F], rec[:], 0.0,
                                        AT.mult, AT.min)
                qe = opool.tile([128, OUT_F], f32, tag="qe")
                nc.scalar.activation(qe[:], xm[:], AF.Exp)
                elu = opool.tile([128, OUT_F], f32, tag="elu")
                nc.vector.scalar_tensor_tensor(elu[:], r1[:], -1.0, qe[:],
                                               AT.add, AT.add)
                nc.sync.dma_start(out=out_ext[t * 128:(t + 1) * 128, :],
                                  in_=elu[:])

    nc.finalize()
    _BUILD_CACHE[key] = nc
    return nc


def kernel(h, adj, W, a1, a2):
    h = np.asarray(h, dtype=np.float32)
    W = np.asarray(W, dtype=np.float32)
    a1 = np.asarray(a1, dtype=np.float32)
    a2 = np.asarray(a2, dtype=np.float32)

    nc = _build_nc()

    hT = np.ascontiguousarray(h.T)
    adj8 = np.asarray(adj, dtype=np.int8)
    Wt = np.ascontiguousarray(W.T)
    a12 = np.ascontiguousarray(np.stack([a1, a2], axis=1))

    in_maps = []
    for c in range(CORES):
        in_maps.append({
            "hT": hT,
            "hTl": np.ascontiguousarray(hT[:, c * R:(c + 1) * R]),
            "adj": adj8[c * R:(c + 1) * R, :],
            "W": W,
            "Wt": Wt,
            "a12": a12,
        })
    res = run_bass_kernel_spmd(nc, in_maps, list(range(CORES)))
    out = np.concatenate([res.results[c]["out"] for c in range(CORES)], axis=0)
    return out
